# revision 29
# baseline (speedup 1.0000x reference)
"""Trainium2 Bass kernel for nn_CoordinateDecoder.

Computation (see reference): posenc(coords) ++ bilinear-pyramid-sampled
features -> 5-layer MLP (gelu-tanh approx, skip concat at depth 2, tanh out).

Strategy (v3, fused sampling):
  - Data-parallel over B: core b handles batch image b (coords/weights shared).
  - KEY IDEA: bilinear sampling commutes with the (linear) layer-0 / layer-3
    weight multiply.  Host transforms each pyramid grid through the matching
    weight slice (tg = grid @ w_level, O(grid) work), and the device sampling
    matmul  Q[bucket]^T @ S  then directly produces the MLP pre-activation
    contribution.  The explicit feature tensor x is never materialized:
        h0_pre = sum_lvl Q0_lvl[bucket]^T S_lvl   (posenc folded into L2 pass)
        h3_pre = w3_h^T h2 + sum_lvl Q3_lvl[bucket]^T S_lvl
    This cuts tensor-engine columns from ~48N to ~22N and removes all
    sampling PSUM->SBUF copies.
  - Samples host-sorted by continuous y; every level's y-buckets are
    contiguous runs.  L0: 2-row pairs (63 buckets, k=128).  L1: 4-row groups
    (11 buckets, k=128).  L2: 4-row groups (5 buckets, k=64) sharing its pass
    with the 42-row posenc block (k=106 total), so the positional encoding
    costs no extra matmul columns.
  - MLP in bf16 (fp32 PSUM), gelu fused on scalar engine over [128,1024]
    2-bank PSUM tiles.  Output layer col-tiled 4-wide on the PE array;
    final tanh + b_out on host (output is [N,3] either way).
  - The out-layer matmuls of super s are emitted after super s+1's layer-0
    matmuls so they never wait on the scalar engine's h3 tail.
"""

import numpy as np
import ml_dtypes

BF16 = ml_dtypes.bfloat16

B, H, W, C = 8, 64, 64, 256
N = 16384
NUM_FREQS = 10
MLP_WIDTH = 256

NSUP = 8            # column supers
SUP = N // NSUP     # 2048
CH = 512            # psum bank quantum (fp32)
NCHUNK = N // CH    # 32 global 512-chunks

NB0 = 63            # L0 row-pair buckets (y0 in [0,62])
NB1 = 11            # L1 4-row buckets (y0//3, y0 in [0,30])
NB2 = 5             # L2 4-row buckets (y0//3, y0 in [0,14])
K2E = 106           # L2+enc pass contraction: 42 posenc + 4*16 grid


def _resize_matrix(out_size: int, in_size: int) -> np.ndarray:
    """Row-resize operator of jax.image.resize(..., 'bilinear') (antialias).
    Returns M [out_size, in_size] with resized = M @ x."""
    scale = out_size / in_size
    inv_scale = 1.0 / scale
    kernel_scale = max(inv_scale, 1.0)
    sample_f = (np.arange(out_size, dtype=np.float64) + 0.5) * inv_scale - 0.5
    x = np.abs(sample_f[None, :] - np.arange(in_size, dtype=np.float64)[:, None])
    x = x / kernel_scale
    w = np.where(x < 1.0, 1.0 - x, 0.0)
    total = w.sum(axis=0, keepdims=True)
    w = np.where(
        np.abs(total) > 1000.0 * np.finfo(np.float32).eps,
        w / np.where(total != 0.0, total, 1.0),
        0.0,
    )
    w = np.where(
        ((sample_f >= -0.5) & (sample_f <= in_size - 0.5))[None, :], w, 0.0
    )
    return w.T.astype(np.float32)  # [out, in]


def _posenc_t(coords: np.ndarray) -> np.ndarray:
    """Transposed positional encoding [42, n] fp32, matching reference order."""
    freqs = (2.0 ** np.arange(NUM_FREQS, dtype=np.float32)) * np.float32(np.pi)
    parts = [coords.T.astype(np.float32)]
    for f in freqs:
        parts.append(np.sin(coords.T * f).astype(np.float32))
        parts.append(np.cos(coords.T * f).astype(np.float32))
    return np.concatenate(parts, axis=0)  # [42, n]


def _bilinear(c01: np.ndarray, size: int):
    """c01 [n] in [0,1] -> (i0, frac) fp32 like the reference's fp32 math."""
    cr = (c01 * np.float32(size - 1)).astype(np.float32)
    i0 = np.floor(cr).astype(np.int64)
    i0 = np.clip(i0, 0, size - 2)
    f = cr - i0.astype(np.float32)
    return i0, f.astype(np.float32)


def _build_runs(bucket: np.ndarray):
    """Maximal constant runs of `bucket` (sorted), split at CH boundaries.
    Returns runs[chunk] = list of (bucket, off_in_chunk, length)."""
    per_chunk = [[] for _ in range(NCHUNK)]
    start = 0
    while start < N:
        g = bucket[start]
        end = start
        while end < N and bucket[end] == g:
            end += 1
        p = start
        while p < end:
            ci = p // CH
            q = min(end, (ci + 1) * CH)
            per_chunk[ci].append((int(g), p - ci * CH, q - p))
            p = q
        start = end
    return per_chunk


def _host_prep(feature_grid, coords, w0, b0, w1, b1, w2, b2, w3, b3, w_out, b_out):
    fg = np.asarray(feature_grid, dtype=np.float32)
    coords = np.asarray(coords, dtype=np.float32)
    w0 = np.asarray(w0, np.float32); w1 = np.asarray(w1, np.float32)
    w2 = np.asarray(w2, np.float32); w3 = np.asarray(w3, np.float32)
    w_out = np.asarray(w_out, np.float32)

    # ---- sort samples by continuous y so every level's y-buckets are runs ----
    c01 = (coords + np.float32(1.0)) / np.float32(2.0)  # [N,2] (y, x)
    perm = np.argsort(c01[:, 0], kind="stable")
    c01s = c01[perm]
    coords_s = coords[perm]

    # ---- per-level bilinear indices / weights ------------------------------
    y0, fy, x0, fx = [], [], [], []
    for S in (64, 32, 16):
        yi, fyi = _bilinear(c01s[:, 0], S)
        xi, fxi = _bilinear(c01s[:, 1], S)
        y0.append(yi); fy.append(fyi); x0.append(xi); fx.append(fxi)

    # ---- buckets -----------------------------------------------------------
    y1g = y0[1] // 3
    dy1 = y0[1] - 3 * y1g
    y2g = y0[2] // 3
    dy2 = y0[2] - 3 * y2g

    runs0 = _build_runs(y0[0])
    runs1 = _build_runs(y1g)
    runs2 = _build_runs(y2g)

    # ---- S matrices: bilinear weights in k-partition layout ----------------
    j = np.arange(N)
    s0 = np.zeros((128, N), np.float32)
    s0[x0[0], j] = (1 - fy[0]) * (1 - fx[0])
    s0[x0[0] + 1, j] = (1 - fy[0]) * fx[0]
    s0[64 + x0[0], j] = fy[0] * (1 - fx[0])
    s0[64 + x0[0] + 1, j] = fy[0] * fx[0]

    s1 = np.zeros((128, N), np.float32)
    s1[dy1 * 32 + x0[1], j] = (1 - fy[1]) * (1 - fx[1])
    s1[dy1 * 32 + x0[1] + 1, j] = (1 - fy[1]) * fx[1]
    s1[(dy1 + 1) * 32 + x0[1], j] = fy[1] * (1 - fx[1])
    s1[(dy1 + 1) * 32 + x0[1] + 1, j] = fy[1] * fx[1]

    s2e = np.zeros((K2E, N), np.float32)
    s2e[0:42] = _posenc_t(coords_s)
    s2e[42 + dy2 * 16 + x0[2], j] = (1 - fy[2]) * (1 - fx[2])
    s2e[42 + dy2 * 16 + x0[2] + 1, j] = (1 - fy[2]) * fx[2]
    s2e[42 + (dy2 + 1) * 16 + x0[2], j] = fy[2] * (1 - fx[2])
    s2e[42 + (dy2 + 1) * 16 + x0[2] + 1, j] = fy[2] * fx[2]

    # ---- pyramid + weight-transformed grids --------------------------------
    R1 = _resize_matrix(32, 64)
    R2 = _resize_matrix(16, 64)
    g1 = np.einsum("ph,qw,bhwc->bpqc", R1, R1, fg, optimize=True)
    g2 = np.einsum("ph,qw,bhwc->bpqc", R2, R2, fg, optimize=True)

    def tgrid(g, ws):  # g [B, s, s, C], ws [C, 256] -> [B, s, s, 256]
        s = g.shape[1]
        return (g.reshape(B * s * s, C) @ ws).reshape(B, s, s, 256)

    tg0a = tgrid(fg, w0[42:298]);  tg0b = tgrid(fg, w3[298:554])
    tg1a = tgrid(g1, w0[298:554]); tg1b = tgrid(g1, w3[554:810])
    tg2a = tgrid(g2, w0[554:810]); tg2b = tgrid(g2, w3[810:1066])
    w0enc = w0[0:42]
    w3enc = w3[256:298]

    # Q tensors interleave the two weight sets per bucket ([a_g | b_g] in one
    # 512-col block) so one staged DMA delivers a bucket range for BOTH the
    # layer-0 and the layer-3 passes.
    def q0_tensor(ta, tb):  # 2x [64,64,256] -> [128, 63*512]
        arr = np.empty((NB0, 128, 512), np.float32)
        for g in range(NB0):
            arr[g, :, 0:256] = ta[g:g + 2].reshape(128, 256)
            arr[g, :, 256:512] = tb[g:g + 2].reshape(128, 256)
        return np.ascontiguousarray(
            arr.transpose(1, 0, 2).reshape(128, NB0 * 512)).astype(BF16)

    def q1_tensor(ta, tb):  # 2x [32,32,256] -> [128, 11*512], 4-row groups
        arr = np.zeros((NB1, 4, 32, 512), np.float32)
        for g in range(NB1):
            rows_a = ta[3 * g:3 * g + 4]
            rows_b = tb[3 * g:3 * g + 4]
            arr[g, :rows_a.shape[0], :, 0:256] = rows_a
            arr[g, :rows_b.shape[0], :, 256:512] = rows_b
        return np.ascontiguousarray(
            arr.reshape(NB1, 128, 512).transpose(1, 0, 2)
            .reshape(128, NB1 * 512)).astype(BF16)

    def q2e_tensor(ta, tb):  # 2x [16,16,256] -> [106, 5*512]
        arr = np.zeros((NB2, K2E, 512), np.float32)
        for g in range(NB2):
            arr[g, 0:42, 0:256] = w0enc
            arr[g, 0:42, 256:512] = w3enc
            arr[g, 42:, 0:256] = ta[3 * g:3 * g + 4].reshape(64, 256)
            arr[g, 42:, 256:512] = tb[3 * g:3 * g + 4].reshape(64, 256)
        return np.ascontiguousarray(
            arr.transpose(1, 0, 2).reshape(K2E, NB2 * 512)).astype(BF16)

    per_core = []
    for b in range(B):
        per_core.append({
            "q0": q0_tensor(tg0a[b], tg0b[b]),
            "q1": q1_tensor(tg1a[b], tg1b[b]),
            "q2e": q2e_tensor(tg2a[b], tg2b[b]),
        })

    # ---- shared tensors ----------------------------------------------------
    def pack(wd):  # [Ktot, M] -> [128, (Ktot/128) * M], k-tile major
        K, M = wd.shape
        return np.ascontiguousarray(
            wd.reshape(K // 128, 128, M).transpose(1, 0, 2).reshape(128, -1)
        )

    # stream tensor: per super [s0 | s1 | s2e (padded to 128 rows)] so each
    # super needs a single 12KB-line DMA
    st_all = np.zeros((128, NSUP * 3 * SUP), np.float32)
    for s in range(NSUP):
        base = s * 3 * SUP
        sl = slice(s * SUP, (s + 1) * SUP)
        st_all[:, base:base + SUP] = s0[:, sl]
        st_all[:, base + SUP:base + 2 * SUP] = s1[:, sl]
        st_all[0:K2E, base + 2 * SUP:base + 3 * SUP] = s2e[:, sl]

    # all small weights in one tensor: w1 | w2 | w3h | wout
    wpack = np.concatenate(
        [pack(w1), pack(w2), pack(w3[0:256]), pack(w_out)], axis=1)
    bias = np.stack([np.asarray(bb, np.float32).reshape(2, 128).T
                     for bb in (b0, b1, b2, b3)], axis=1).reshape(128, 8)

    shared = {
        "st_all": st_all.astype(BF16),
        "wpack": wpack.astype(BF16),
        "bias": np.ascontiguousarray(bias),
    }
    return shared, per_core, perm, runs0, runs1, runs2


_DRAM_SPECS = [
    ("q0", (128, NB0 * 512), BF16),
    ("q1", (128, NB1 * 512), BF16),
    ("q2e", (K2E, NB2 * 512), BF16),
    ("st_all", (128, NSUP * 3 * SUP), BF16),
    ("wpack", (128, 3 * 512 + 6), BF16),
    ("bias", (128, 8), np.float32),
]


def _build_nc(runs0, runs1, runs2):
    """Build the Bacc program (shared by all cores; per-core data differs)."""
    from contextlib import ExitStack

    import concourse.bacc as bacc
    import concourse.mybir as mybir
    import concourse.tile as tile

    bf16 = mybir.dt.bfloat16
    f32 = mybir.dt.float32
    GELU = mybir.ActivationFunctionType.Gelu_apprx_tanh

    nc = bacc.Bacc("TRN2", debug=False, target_bir_lowering=False)

    dram = {}
    for name, shape, npdt in _DRAM_SPECS:
        dram[name] = nc.dram_tensor(
            name, list(shape), mybir.dt.from_np(np.dtype(npdt)), kind="ExternalInput"
        )
    out_dram = nc.dram_tensor("out_t", [3, N], f32, kind="ExternalOutput")

    with tile.TileContext(nc) as tc, ExitStack() as ctx:
        const = ctx.enter_context(tc.tile_pool(name="const", bufs=1))
        spool = ctx.enter_context(tc.tile_pool(name="stream", bufs=2))
        hpool = ctx.enter_context(tc.tile_pool(name="h", bufs=1))
        opool = ctx.enter_context(tc.tile_pool(name="osb", bufs=2))
        ps_mlp = ctx.enter_context(tc.tile_pool(name="ps_mlp", bufs=3, space="PSUM"))
        ps_out = ctx.enter_context(tc.tile_pool(name="ps_out", bufs=2, space="PSUM"))

        st = {}
        sdict = {n: (s, d) for n, s, d in _DRAM_SPECS}
        for name in ("bias", "wpack"):
            shape, npdt = sdict[name]
            t = const.tile(list(shape), mybir.dt.from_np(np.dtype(npdt)), tag=name)
            nc.sync.dma_start(t[:, :], dram[name][:, :])
            st[name] = t
        for name in ("q0", "q1", "q2e"):
            shape, npdt = sdict[name]
            st[name] = const.tile(
                list(shape), mybir.dt.from_np(np.dtype(npdt)), tag=name,
                name=name)
        wp = st["wpack"]
        wmlp = {"w1": wp[:, 0:512], "w2": wp[:, 512:1024],
                "w3h": wp[:, 1024:1536]}
        wout = wp[:, 1536:1542]

        def _qdma(name, blo, bhi):
            nc.sync.dma_start(st[name][:, blo * 512:bhi * 512],
                              dram[name][:, blo * 512:bhi * 512])

        # Q-tensor DMAs staged across supers by bucket-block range, ordered
        # by first use (super s touches L0 buckets <= ~7.9(s+1)).  Transfers
        # complete in emission order, so stage s only ships what supers s and
        # s+1 are about to read.
        qdma_stage = {
            1: [("q0", 9, 18), ("q1", 2, 3), ("q2e", 1, 2)],
            2: [("q0", 18, 34), ("q1", 3, 6), ("q2e", 2, 3)],
            3: [("q0", 34, 50), ("q1", 6, 9), ("q2e", 3, 4)],
            4: [("q0", 50, 63), ("q1", 9, 11), ("q2e", 4, 5)],
        }

        bias = st["bias"]
        prev = None  # (h3 tile, super index) pending output stage

        def out_half(h3t, s_idx, half):
            # two chunks col-tiled to PE col-groups 0/1 (concurrent streams)
            po = ps_out.tile([128, CH], f32, tag="po", name="po")
            for kt in range(2):
                for jq in range(2):
                    ci = 2 * half + jq
                    nc.tensor.matmul(
                        po[32 * jq:32 * jq + 3, :],
                        wout[:, kt * 3:(kt + 1) * 3],
                        h3t[:, kt * SUP + ci * CH: kt * SUP + ci * CH + CH],
                        start=(kt == 0), stop=(kt == 1),
                        tile_position=(0, 32 * jq),
                    )
            ob = opool.tile([128, CH], f32, tag="ob", name="ob")
            nc.vector.tensor_copy(ob[0:64, :], po[0:64, :])
            for jq in range(2):
                lo = s_idx * SUP + (2 * half + jq) * CH
                nc.sync.dma_start(out_dram[:, lo:lo + CH], ob[32 * jq:32 * jq + 3, :])

        def out_stage(h3t, s_idx):
            out_half(h3t, s_idx, 0)
            out_half(h3t, s_idx, 1)

        for s in range(NSUP):
            stile = spool.tile([128, 3 * SUP], bf16, tag="stile")
            base_c = s * 3 * SUP
            if s == 0:
                # fine-grained first super: land the first compute chunk's
                # inputs (t=0 halves of s0/s1/s2e + the first L0 blocks)
                # before the bulk, so the PE starts ~5us earlier
                for sec in range(3):
                    nc.sync.dma_start(
                        stile[:, sec * SUP:sec * SUP + 1024],
                        dram["st_all"][:, base_c + sec * SUP:
                                       base_c + sec * SUP + 1024])
                _qdma("q0", 0, 3)
                _qdma("q1", 0, 1)
                _qdma("q2e", 0, 1)
                for sec in range(3):
                    nc.sync.dma_start(
                        stile[:, sec * SUP + 1024:(sec + 1) * SUP],
                        dram["st_all"][:, base_c + sec * SUP + 1024:
                                       base_c + (sec + 1) * SUP])
                _qdma("q0", 3, 9)
                _qdma("q1", 1, 2)
            else:
                nc.sync.dma_start(stile[:, :],
                                  dram["st_all"][:, base_c:base_c + 3 * SUP])
                for name, blo, bhi in qdma_stage.get(s, ()):
                    _qdma(name, blo, bhi)
            s0 = stile[:, 0:SUP]
            s1 = stile[:, SUP:2 * SUP]
            s2e = stile[0:K2E, 2 * SUP:3 * SUP]

            def samp_pass(ps, m, t, base, first_start):
                """Accumulate L0 + L1 + (L2+enc) contributions into ps
                [128,1024] (psum-tile t of this super, output m-tile m).
                base selects the weight set: 0 = w0 (h0), 256 = w3 (h3)."""
                for c in range(2):
                    gc = s * 4 + t * 2 + c          # global 512-chunk
                    col = t * 1024 + c * 512        # column base in super
                    po_ = c * 512                   # column base in ps tile
                    first = first_start
                    for (g, off, ln) in runs0[gc]:
                        nc.tensor.matmul(
                            ps[:, po_ + off:po_ + off + ln],
                            st["q0"][:, g * 512 + base + m * 128:
                                     g * 512 + base + m * 128 + 128],
                            s0[:, col + off: col + off + ln],
                            start=first, stop=False,
                        )
                        first = False
                    for (g, off, ln) in runs1[gc]:
                        nc.tensor.matmul(
                            ps[:, po_ + off:po_ + off + ln],
                            st["q1"][:, g * 512 + base + m * 128:
                                     g * 512 + base + m * 128 + 128],
                            s1[:, col + off: col + off + ln],
                            start=False, stop=False,
                        )
                    for (g, off, ln) in runs2[gc]:
                        nc.tensor.matmul(
                            ps[:, po_ + off:po_ + off + ln],
                            st["q2e"][:, g * 512 + base + m * 128:
                                      g * 512 + base + m * 128 + 128],
                            s2e[:, col + off: col + off + ln],
                            start=False, stop=True,
                        )

            # ---- layer 0 ----------------------------------------------------
            h0 = hpool.tile([128, 2 * SUP], bf16, tag="h0")
            for t in range(2):
                for m in range(2):
                    ps = ps_mlp.tile([128, 1024], f32, tag="ps")
                    samp_pass(ps, m, t, 0, True)
                    nc.scalar.activation(
                        h0[:, m * SUP + t * 1024: m * SUP + t * 1024 + 1024],
                        ps[:, :], GELU, bias=bias[:, m:m + 1],
                    )

            # out stage of the previous super runs here: its h3 activations
            # are complete by now, so the PE never waits on the scalar tail.
            if prev is not None:
                out_stage(*prev)

            # ---- layers 1, 2 (dense 256x256) -------------------------------
            def dense(layer, wname, hin, tag):
                h = hpool.tile([128, 2 * SUP], bf16, tag=tag, name=tag)
                for t in range(2):
                    for m in range(2):
                        ps = ps_mlp.tile([128, 1024], f32, tag="ps")
                        for kt in range(2):
                            lhsT = wmlp[wname][:, kt * 256 + m * 128:
                                               kt * 256 + m * 128 + 128]
                            for c in range(2):
                                nc.tensor.matmul(
                                    ps[:, c * 512:c * 512 + 512],
                                    lhsT,
                                    hin[:, kt * SUP + t * 1024 + c * 512:
                                        kt * SUP + t * 1024 + c * 512 + 512],
                                    start=(kt == 0), stop=(kt == 1),
                                )
                        nc.scalar.activation(
                            h[:, m * SUP + t * 1024: m * SUP + t * 1024 + 1024],
                            ps[:, :], GELU, bias=bias[:, 2 * layer + m:
                                                      2 * layer + m + 1],
                        )
                return h

            h1 = dense(1, "w1", h0, "h1")
            h2 = dense(2, "w2", h1, "h2")

            # ---- layer 3: w3_h^T h2 + skip (enc + levels via w3) -----------
            h3 = hpool.tile([128, 2 * SUP], bf16, tag="h3", bufs=2)
            for t in range(2):
                for m in range(2):
                    ps = ps_mlp.tile([128, 1024], f32, tag="ps")
                    for kt in range(2):
                        lhsT = wmlp["w3h"][:, kt * 256 + m * 128:
                                           kt * 256 + m * 128 + 128]
                        for c in range(2):
                            nc.tensor.matmul(
                                ps[:, c * 512:c * 512 + 512],
                                lhsT,
                                h2[:, kt * SUP + t * 1024 + c * 512:
                                    kt * SUP + t * 1024 + c * 512 + 512],
                                start=(kt == 0), stop=False,
                            )
                    samp_pass(ps, m, t, 256, False)
                    nc.scalar.activation(
                        h3[:, m * SUP + t * 1024: m * SUP + t * 1024 + 1024],
                        ps[:, :], GELU, bias=bias[:, 6 + m:7 + m],
                    )
            if s == NSUP - 1:
                # last super: both out halves after all h3 matmuls — half 0's
                # activations complete while the t=1 matmuls stream, so only
                # half 1 waits on the scalar tail
                out_half(h3, s, 0)
                out_half(h3, s, 1)
            else:
                prev = (h3, s)

    nc.compile()
    return nc


def kernel(feature_grid, coords, w0, b0, w1, b1, w2, b2, w3, b3, w_out, b_out,
           _run_opts=None):
    from concourse.bass_utils import run_bass_kernel_spmd

    shared, per_core, perm, runs0, runs1, runs2 = _host_prep(
        feature_grid, coords, w0, b0, w1, b1, w2, b2, w3, b3, w_out, b_out)

    nc = _build_nc(runs0, runs1, runs2)

    in_maps = []
    for b in range(B):
        m = dict(shared)
        m.update(per_core[b])
        in_maps.append(m)

    res = run_bass_kernel_spmd(
        nc, in_maps, core_ids=list(range(B)), **(_run_opts or {})
    )

    bout = np.asarray(b_out, np.float32).reshape(1, 3)
    out = np.empty((B, N, 3), np.float32)
    for b in range(B):
        out[b, perm, :] = np.tanh(res.results[b]["out_t"].T + bout)
    if _run_opts is not None:
        kernel._last_result = res  # for test harness introspection
    return out
